# revision 22
# baseline (speedup 1.0000x reference)
"""BitLinear (BitNet b1.58) forward kernel for Trainium2, 8 NeuronCores.

y = act_quant(x) @ weight_quant(W)^T + bias
  - activation quant: per-token absmax int8 fake-quant (values in [-127,127])
  - weight quant: per-tensor mean-absmax ternary fake-quant {-1,0,1}

Sharding: data-parallel over the batch dim (8 batches -> 1 per core);
W and bias are replicated per core, each core computes mean(|W|) locally
(no collectives needed).

Dataflow (v2): the PE does ONLY the 512 N=512 fp16 matmuls.
  * Output is computed TRANSPOSED (yT[out, token]): stationary operand =
    quantized weights wq[din, out] fp16 = ternary*v_w (host passes W^T so
    no W transposes), moving operand = dequantized activations
    dqT[din, token] fp16 = round(x*s)*fl(1/s).
  * dq tiles [128 tok, 512 din] are transposed to [din, tok] by the DMA
    xbar (one 3D-output transpose DMA per tile; out[p,k,t] = in[t,128k+p]).
  * mean(|W|) uses the baseline's exact split summation (the nearest
    weight sits ~2.6e-7 from a ternary rounding boundary); everything
    downstream only needs the 2e-2 harness tolerance, which fp16 I/O
    satisfies with ~10x margin.

Per 512-token group: 4 x-tile loads (gpsimd SWDGE), per-token absmax ->
scales (DVE), round pass (ACT), dequant+fp16 pass (DVE), xbar transpose
(sync HWDGE), 16x4 matmuls into 8 rotating PSUM banks (PE), 16 bias
epilogues (ACT Identity+bias vs DVE tensor_scalar add), 2 half-slab
yT stores (sync HWDGE).
"""

import os
import sys

import numpy as np

B, S, DIN, DOUT = 8, 4096, 512, 2048
N_CORES = 8
KC = DIN // 128          # 4 contraction chunks
MC = DOUT // 128         # 16 output chunks
GTOK = 512               # tokens per group (matmul N / PSUM bank size)
GROUPS = S // GTOK       # 8 groups per core
TPG = GTOK // 128        # 4 token tiles per group

MAGIC = 12582912.0   # 1.5 * 2^23: (v + MAGIC) - MAGIC == round-half-even(v), |v| < 2^22
C_GRID_11 = 6144.0   # 1.5 * 2^12: rounds to multiples of 2^-11 (values <= ~2048)
C_GRID_4 = 786432.0  # 1.5 * 2^19: rounds to multiples of 2^-4  (values <= ~2^18)
EPS = 1e-6

_cached = {}


def _ensure_path():
    try:
        import concourse  # noqa: F401
    except ImportError:
        for p in ("/opt/trn_rl_repo", os.path.expanduser("~/.axon_site/_ro/trn_rl_repo")):
            if os.path.isdir(p) and p not in sys.path:
                sys.path.insert(0, p)


def build_program():
    _ensure_path()
    from contextlib import ExitStack

    import concourse.bacc as bacc
    import concourse.tile as tile
    from concourse import mybir

    f32 = mybir.dt.float32
    f16 = mybir.dt.float16
    Alu = mybir.AluOpType
    X = mybir.AxisListType.X
    Copy = mybir.ActivationFunctionType.Copy
    Ident = mybir.ActivationFunctionType.Identity

    nc = bacc.Bacc("TRN2", target_bir_lowering=False, debug=False, num_devices=N_CORES)
    x_d = nc.dram_tensor("x", [S, DIN], f32, kind="ExternalInput").ap()
    # host passes W^T chunked: wt[p, k, :] = W.T[k*128 + p, :]
    wt_d = nc.dram_tensor("wt", [128, KC, DOUT], f32, kind="ExternalInput").ap()
    b_d = nc.dram_tensor("bias", [DOUT], f32, kind="ExternalInput").ap()
    yT_d = nc.dram_tensor("yT", [DOUT, S], f16, kind="ExternalOutput").ap()
    # yT viewed as [p, m, t]: out feature m*128+p, token t
    yT_r = yT_d.rearrange("(m p) t -> p m t", p=128)

    with tile.TileContext(nc) as tc, ExitStack() as ctx:
        cpool = ctx.enter_context(tc.tile_pool(name="const", bufs=1))
        wtp = ctx.enter_context(tc.tile_pool(name="wt", bufs=1))
        wqp = ctx.enter_context(tc.tile_pool(name="wq", bufs=1))
        wrp = ctx.enter_context(tc.tile_pool(name="wtmp", bufs=3))
        statp = ctx.enter_context(tc.tile_pool(name="stat", bufs=1))
        xp = ctx.enter_context(tc.tile_pool(name="x", bufs=5))
        mxp = ctx.enter_context(tc.tile_pool(name="mx", bufs=16))
        r1p = ctx.enter_context(tc.tile_pool(name="r1", bufs=6))
        dqp = ctx.enter_context(tc.tile_pool(name="dq", bufs=6))
        dqTp = ctx.enter_context(tc.tile_pool(name="dqT", bufs=3))
        yp = ctx.enter_context(tc.tile_pool(name="y", bufs=2))
        psp = ctx.enter_context(tc.tile_pool(name="ps", bufs=8, space="PSUM"))

        # ---- constants ----
        bias_col = cpool.tile([128, MC], f32)
        nc.scalar.dma_start(bias_col[:], b_d.rearrange("(m p) -> p m", p=128))
        ones128 = cpool.tile([128, 128], f32)
        nc.vector.memset(ones128[:], 1.0)

        # ---- W load: per-chunk DMAs so abs-sums start as chunks land ----
        wt_sb = wtp.tile([128, KC, DOUT], f32)
        for k in range(KC):
            nc.sync.dma_start(wt_sb[:, k : k + 1, :], wt_d[:, k : k + 1, :])

        # ---- mean(|W|): exact-split summation (see baseline notes) ----
        wsum = statp.tile([128, KC], f32)
        for k in range(KC):
            nc.vector.tensor_reduce(
                wsum[:, k : k + 1], wt_sb[:, k, :],
                axis=X, op=Alu.add, apply_absolute_value=True,
            )
        hh = statp.tile([128, KC], f32)
        ll = statp.tile([128, KC], f32)
        nc.vector.tensor_scalar(hh[:], wsum[:], C_GRID_11, C_GRID_11, op0=Alu.add, op1=Alu.subtract)
        nc.vector.tensor_tensor(ll[:], wsum[:], hh[:], op=Alu.subtract)
        hs = statp.tile([128, 1], f32)
        ls = statp.tile([128, 1], f32)
        nc.vector.tensor_reduce(hs[:], hh[:], axis=X, op=Alu.add)
        nc.vector.tensor_reduce(ls[:], ll[:], axis=X, op=Alu.add)
        red = statp.tile([128, 2], f32)
        l2 = statp.tile([128, 1], f32)
        nc.vector.tensor_scalar(red[:, 0:1], hs[:], C_GRID_4, C_GRID_4, op0=Alu.add, op1=Alu.subtract)
        nc.vector.tensor_tensor(l2[:], hs[:], red[:, 0:1], op=Alu.subtract)
        nc.vector.tensor_tensor(red[:, 1:2], l2[:], ls[:], op=Alu.add)
        # cross-partition sum + broadcast in one exact fp32 ones-matmul
        pred = psp.tile([128, 2], f32, tag="mm", name="pred")
        nc.tensor.matmul(pred[:], ones128[:], red[:], start=True, stop=True)
        redo = statp.tile([128, 2], f32)
        nc.scalar.copy(redo[:], pred[:])
        ssum = statp.tile([128, 1], f32)
        nc.vector.tensor_tensor(ssum[:], redo[:, 0:1], redo[:, 1:2], op=Alu.add)
        mean_t = statp.tile([128, 1], f32)
        nc.vector.tensor_scalar(mean_t[:], ssum[:], 1.0 / (DOUT * DIN), None, op0=Alu.mult)
        nc.vector.tensor_scalar(mean_t[:], mean_t[:], EPS, None, op0=Alu.max)
        s_w = statp.tile([128, 1], f32)   # 1/mean: the quantization scale
        nc.vector.reciprocal(s_w[:], mean_t[:])
        # dequant magnitude: ref uses fl(1/s_w), which is within 2 ulp of
        # mean itself — a uniform ~1e-7 output scale difference, far inside
        # the 2e-2 gate. Using mean_t drops a serial DVE op (and its queue
        # delay) from the critical path to the first matmul.
        v_w = mean_t

        # ---- W quantize into wq fp16 = ternary * v_w, m-major so the first
        # out-chunks are ready earliest ----
        wq = wqp.tile([128, KC, DOUT], f16)
        for q4 in range(4):
            sl = slice(q4 * 512, (q4 + 1) * 512)
            for k in range(KC):
                wr = wrp.tile([128, 512], f32, tag="wr")
                nc.scalar.activation(wr[:], wt_sb[:, k, sl], Copy, bias=MAGIC, scale=s_w[:])
                wq2 = wrp.tile([128, 512], f32, tag="wq2")
                nc.vector.tensor_scalar(wq2[:], wr[:], MAGIC, 1.0, op0=Alu.subtract, op1=Alu.min)
                nc.vector.tensor_scalar(wq[:, k, sl], wq2[:], -1.0, v_w[:], op0=Alu.max, op1=Alu.mult)

        # ---- main loop over 512-token groups ----
        for g in range(GROUPS):
            dqT = dqTp.tile([128, KC, GTOK], f16)
            for t in range(TPG):
                i = g * TPG + t
                xt = xp.tile([128, DIN], f32)
                nc.gpsimd.dma_start(xt[:], x_d[i * 128 : (i + 1) * 128, :])
                mx = mxp.tile([128, 1], f32, tag="mx")
                nc.vector.tensor_reduce(mx[:], xt[:], axis=X, op=Alu.max, apply_absolute_value=True)
                s = mxp.tile([128, 1], f32, tag="s")
                nc.vector.reciprocal(s[:], mx[:])
                nc.vector.tensor_scalar(s[:], s[:], 127.0, None, op0=Alu.mult)
                f = mxp.tile([128, 1], f32, tag="f")
                nc.vector.reciprocal(f[:], s[:])
                r1 = r1p.tile([128, DIN], f32)
                nc.scalar.activation(r1[:], xt[:], Copy, bias=MAGIC, scale=s[:])
                dq = dqp.tile([128, DIN], f16)
                nc.vector.tensor_scalar(dq[:], r1[:], MAGIC, f[:], op0=Alu.subtract, op1=Alu.mult)
                # xbar transpose: dqT[p, k, t*128+tt] = dq[tt, k*128+p]
                nc.sync.dma_start(dqT[:, :, t * 128 : (t + 1) * 128], dq[:], transpose=True)

            ysb = yp.tile([128, MC, GTOK], f16)
            for m in range(MC):
                pm = psp.tile([128, GTOK], f32, tag="mm", name=f"pm{g}_{m}")
                for k in range(KC):
                    nc.tensor.matmul(
                        pm[:], wq[:, k, m * 128 : (m + 1) * 128], dqT[:, k, :],
                        start=(k == 0), stop=(k == KC - 1),
                    )
                # epilogue engine split: in groups 0-3 the DVE still carries
                # the W-quant burst, so give it fewer epilogues there
                on_dve = (m % 4 == 2) if g < 4 else (m % 2 == 1)
                if on_dve:
                    nc.vector.tensor_scalar(
                        ysb[:, m, :], pm[:], bias_col[:, m : m + 1], None, op0=Alu.add,
                    )
                else:
                    nc.scalar.activation(
                        ysb[:, m, :], pm[:], Ident,
                        bias=bias_col[:, m : m + 1], scale=1.0,
                    )
            sl_t = slice(g * GTOK, (g + 1) * GTOK)
            nc.sync.dma_start(yT_r[:, 0:8, sl_t], ysb[:, 0:8, :])
            nc.sync.dma_start(yT_r[:, 8:16, sl_t], ysb[:, 8:16, :])

    nc.compile()
    return nc


def _get_program():
    if "nc" not in _cached:
        _cached["nc"] = build_program()
    return _cached["nc"]


def kernel(x: np.ndarray, weight: np.ndarray, bias: np.ndarray) -> np.ndarray:
    _ensure_path()
    from concourse.bass_utils import run_bass_kernel_spmd

    x = np.ascontiguousarray(x, dtype=np.float32)
    weight = np.ascontiguousarray(weight, dtype=np.float32)
    bias = np.ascontiguousarray(bias, dtype=np.float32)
    # wt[p, k, :] = W.T[k*128 + p, :]
    wt = np.ascontiguousarray(weight.T.reshape(KC, 128, DOUT).transpose(1, 0, 2))

    nc = _get_program()
    in_maps = [{"x": x[c], "wt": wt, "bias": bias} for c in range(N_CORES)]
    res = run_bass_kernel_spmd(nc, in_maps, core_ids=list(range(N_CORES)))
    _cached["last_results"] = res
    y = np.empty((B, S, DOUT), dtype=np.float32)
    for c in range(N_CORES):
        y[c] = res.results[c]["yT"].T
    return y


# revision 24
# speedup vs baseline: 1.1298x; 1.1298x over previous
"""BitLinear (BitNet b1.58) forward kernel for Trainium2, 8 NeuronCores.

y = act_quant(x) @ weight_quant(W)^T + bias
  - activation quant: per-token absmax int8 fake-quant (values in [-127,127])
  - weight quant: per-tensor mean-absmax ternary fake-quant {-1,0,1}

Sharding: data-parallel over the batch dim (8 batches -> 1 per core);
W and bias are replicated per core, each core computes mean(|W|) locally
(no collectives needed).

Dataflow (v2): the PE does ONLY the 512 N=512 fp16 matmuls.
  * Output is computed TRANSPOSED (yT[out, token]): stationary operand =
    quantized weights wq[din, out] fp16 = ternary*v_w (host passes W^T so
    no W transposes), moving operand = dequantized activations
    dqT[din, token] fp16 = round(x*s)*fl(1/s).
  * dq tiles [128 tok, 512 din] are transposed to [din, tok] by the DMA
    xbar (one 3D-output transpose DMA per tile; out[p,k,t] = in[t,128k+p]).
  * mean(|W|) uses the baseline's exact split summation (the nearest
    weight sits ~2.6e-7 from a ternary rounding boundary); everything
    downstream only needs the 2e-2 harness tolerance, which fp16 I/O
    satisfies with ~10x margin.

Per 512-token group: 4 x-tile loads (gpsimd SWDGE), per-token absmax ->
scales (DVE), round pass (ACT), dequant+fp16 pass (DVE), xbar transpose
(sync HWDGE), 16x4 matmuls into 8 rotating PSUM banks (PE), 16 bias
epilogues (ACT Identity+bias vs DVE tensor_scalar add), 2 half-slab
yT stores (sync HWDGE).
"""

import os
import sys

import numpy as np

B, S, DIN, DOUT = 8, 4096, 512, 2048
N_CORES = 8
KC = DIN // 128          # 4 contraction chunks
MC = DOUT // 128         # 16 output chunks
GTOK = 512               # tokens per group (matmul N / PSUM bank size)
GROUPS = S // GTOK       # 8 groups per core
TPG = GTOK // 128        # 4 token tiles per group

MAGIC = 12582912.0   # 1.5 * 2^23: (v + MAGIC) - MAGIC == round-half-even(v), |v| < 2^22
C_GRID_11 = 6144.0   # 1.5 * 2^12: rounds to multiples of 2^-11 (values <= ~2048)
C_GRID_4 = 786432.0  # 1.5 * 2^19: rounds to multiples of 2^-4  (values <= ~2^18)
EPS = 1e-6

_cached = {}


def _ensure_path():
    try:
        import concourse  # noqa: F401
    except ImportError:
        for p in ("/opt/trn_rl_repo", os.path.expanduser("~/.axon_site/_ro/trn_rl_repo")):
            if os.path.isdir(p) and p not in sys.path:
                sys.path.insert(0, p)


def build_program():
    _ensure_path()
    from contextlib import ExitStack

    import concourse.bacc as bacc
    import concourse.tile as tile
    from concourse import mybir

    f32 = mybir.dt.float32
    f16 = mybir.dt.float16
    Alu = mybir.AluOpType
    X = mybir.AxisListType.X
    Copy = mybir.ActivationFunctionType.Copy
    Ident = mybir.ActivationFunctionType.Identity

    nc = bacc.Bacc("TRN2", target_bir_lowering=False, debug=False, num_devices=N_CORES)
    x_d = nc.dram_tensor("x", [S, DIN], f32, kind="ExternalInput").ap()
    # host passes W^T chunked: wt[p, k, :] = W.T[k*128 + p, :]
    wt_d = nc.dram_tensor("wt", [128, KC, DOUT], f32, kind="ExternalInput").ap()
    b_d = nc.dram_tensor("bias", [DOUT], f32, kind="ExternalInput").ap()
    yT_d = nc.dram_tensor("yT", [DOUT, S], f16, kind="ExternalOutput").ap()
    # yT viewed as [p, m, t]: out feature m*128+p, token t
    yT_r = yT_d.rearrange("(m p) t -> p m t", p=128)

    with tile.TileContext(nc) as tc, ExitStack() as ctx:
        cpool = ctx.enter_context(tc.tile_pool(name="const", bufs=1))
        wtp = ctx.enter_context(tc.tile_pool(name="wt", bufs=1))
        wqp = ctx.enter_context(tc.tile_pool(name="wq", bufs=1))
        wrp = ctx.enter_context(tc.tile_pool(name="wtmp", bufs=3))
        statp = ctx.enter_context(tc.tile_pool(name="stat", bufs=1))
        xp = ctx.enter_context(tc.tile_pool(name="x", bufs=8))
        mxp = ctx.enter_context(tc.tile_pool(name="mx", bufs=16))
        r1p = ctx.enter_context(tc.tile_pool(name="r1", bufs=6))
        dqp = ctx.enter_context(tc.tile_pool(name="dq", bufs=6))
        dqTp = ctx.enter_context(tc.tile_pool(name="dqT", bufs=3))
        yp = ctx.enter_context(tc.tile_pool(name="y", bufs=2))
        psp = ctx.enter_context(tc.tile_pool(name="ps", bufs=8, space="PSUM"))

        # ---- constants ----
        bias_col = cpool.tile([128, MC], f32)
        nc.scalar.dma_start(bias_col[:], b_d.rearrange("(m p) -> p m", p=128))
        ones128 = cpool.tile([128, 128], f32)
        nc.vector.memset(ones128[:], 1.0)

        # ---- W load: per-chunk DMAs so abs-sums start as chunks land ----
        wt_sb = wtp.tile([128, KC, DOUT], f32)
        for k in range(KC):
            nc.sync.dma_start(wt_sb[:, k : k + 1, :], wt_d[:, k : k + 1, :])

        # ---- mean(|W|): exact-split summation (see baseline notes) ----
        wsum = statp.tile([128, KC], f32)
        for k in range(KC):
            nc.vector.tensor_reduce(
                wsum[:, k : k + 1], wt_sb[:, k, :],
                axis=X, op=Alu.add, apply_absolute_value=True,
            )
        hh = statp.tile([128, KC], f32)
        ll = statp.tile([128, KC], f32)
        nc.vector.tensor_scalar(hh[:], wsum[:], C_GRID_11, C_GRID_11, op0=Alu.add, op1=Alu.subtract)
        nc.vector.tensor_tensor(ll[:], wsum[:], hh[:], op=Alu.subtract)
        hs = statp.tile([128, 1], f32)
        ls = statp.tile([128, 1], f32)
        nc.vector.tensor_reduce(hs[:], hh[:], axis=X, op=Alu.add)
        nc.vector.tensor_reduce(ls[:], ll[:], axis=X, op=Alu.add)
        red = statp.tile([128, 2], f32)
        l2 = statp.tile([128, 1], f32)
        nc.vector.tensor_scalar(red[:, 0:1], hs[:], C_GRID_4, C_GRID_4, op0=Alu.add, op1=Alu.subtract)
        nc.vector.tensor_tensor(l2[:], hs[:], red[:, 0:1], op=Alu.subtract)
        nc.vector.tensor_tensor(red[:, 1:2], l2[:], ls[:], op=Alu.add)
        # cross-partition sum + broadcast in one exact fp32 ones-matmul
        pred = psp.tile([128, 2], f32, tag="mm", name="pred")
        nc.tensor.matmul(pred[:], ones128[:], red[:], start=True, stop=True)
        redo = statp.tile([128, 2], f32)
        nc.scalar.copy(redo[:], pred[:])
        ssum = statp.tile([128, 1], f32)
        nc.vector.tensor_tensor(ssum[:], redo[:, 0:1], redo[:, 1:2], op=Alu.add)
        mean_t = statp.tile([128, 1], f32)
        nc.vector.tensor_scalar(mean_t[:], ssum[:], 1.0 / (DOUT * DIN), None, op0=Alu.mult)
        nc.vector.tensor_scalar(mean_t[:], mean_t[:], EPS, None, op0=Alu.max)
        s_w = statp.tile([128, 1], f32)   # 1/mean: the quantization scale
        nc.vector.reciprocal(s_w[:], mean_t[:])
        v_w = statp.tile([128, 1], f32)   # fl(1/s_w): dequant magnitude (matches ref)
        nc.vector.reciprocal(v_w[:], s_w[:])

        # ---- W quantize into wq fp16 = ternary * v_w, m-major so the first
        # out-chunks are ready earliest ----
        wq = wqp.tile([128, KC, DOUT], f16)
        for q4 in range(4):
            sl = slice(q4 * 512, (q4 + 1) * 512)
            for k in range(KC):
                wr = wrp.tile([128, 512], f32, tag="wr")
                nc.scalar.activation(wr[:], wt_sb[:, k, sl], Copy, bias=MAGIC, scale=s_w[:])
                wq2 = wrp.tile([128, 512], f32, tag="wq2")
                nc.vector.tensor_scalar(wq2[:], wr[:], MAGIC, 1.0, op0=Alu.subtract, op1=Alu.min)
                nc.vector.tensor_scalar(wq[:, k, sl], wq2[:], -1.0, v_w[:], op0=Alu.max, op1=Alu.mult)

        # ---- main loop over 512-token groups ----
        for g in range(GROUPS):
            dqT = dqTp.tile([128, KC, GTOK], f16)
            for t in range(TPG):
                i = g * TPG + t
                xt = xp.tile([128, DIN], f32)
                nc.gpsimd.dma_start(xt[:], x_d[i * 128 : (i + 1) * 128, :])
                mx = mxp.tile([128, 1], f32, tag="mx")
                nc.vector.tensor_reduce(mx[:], xt[:], axis=X, op=Alu.max, apply_absolute_value=True)
                s = mxp.tile([128, 1], f32, tag="s")
                nc.vector.reciprocal(s[:], mx[:])
                nc.vector.tensor_scalar(s[:], s[:], 127.0, None, op0=Alu.mult)
                f = mxp.tile([128, 1], f32, tag="f")
                nc.vector.reciprocal(f[:], s[:])
                r1 = r1p.tile([128, DIN], f32)
                nc.scalar.activation(r1[:], xt[:], Copy, bias=MAGIC, scale=s[:])
                dq = dqp.tile([128, DIN], f16)
                nc.vector.tensor_scalar(dq[:], r1[:], MAGIC, f[:], op0=Alu.subtract, op1=Alu.mult)
                # xbar transpose: dqT[p, k, t*128+tt] = dq[tt, k*128+p]
                nc.sync.dma_start(dqT[:, :, t * 128 : (t + 1) * 128], dq[:], transpose=True)

            ysb = yp.tile([128, MC, GTOK], f16)
            for m in range(MC):
                pm = psp.tile([128, GTOK], f32, tag="mm", name=f"pm{g}_{m}")
                for k in range(KC):
                    nc.tensor.matmul(
                        pm[:], wq[:, k, m * 128 : (m + 1) * 128], dqT[:, k, :],
                        start=(k == 0), stop=(k == KC - 1),
                    )
                # epilogue engine split: in groups 0-3 the DVE still carries
                # the W-quant burst, so give it fewer epilogues there
                on_dve = (m % 4 == 2) if g < 4 else (m % 2 == 1)
                if on_dve:
                    nc.vector.tensor_scalar(
                        ysb[:, m, :], pm[:], bias_col[:, m : m + 1], None, op0=Alu.add,
                    )
                else:
                    nc.scalar.activation(
                        ysb[:, m, :], pm[:], Ident,
                        bias=bias_col[:, m : m + 1], scale=1.0,
                    )
            sl_t = slice(g * GTOK, (g + 1) * GTOK)
            nc.sync.dma_start(yT_r[:, 0:8, sl_t], ysb[:, 0:8, :])
            nc.sync.dma_start(yT_r[:, 8:16, sl_t], ysb[:, 8:16, :])

    nc.compile()
    return nc


def _get_program():
    if "nc" not in _cached:
        _cached["nc"] = build_program()
    return _cached["nc"]


def kernel(x: np.ndarray, weight: np.ndarray, bias: np.ndarray) -> np.ndarray:
    _ensure_path()
    from concourse.bass_utils import run_bass_kernel_spmd

    x = np.ascontiguousarray(x, dtype=np.float32)
    weight = np.ascontiguousarray(weight, dtype=np.float32)
    bias = np.ascontiguousarray(bias, dtype=np.float32)
    # wt[p, k, :] = W.T[k*128 + p, :]
    wt = np.ascontiguousarray(weight.T.reshape(KC, 128, DOUT).transpose(1, 0, 2))

    nc = _get_program()
    in_maps = [{"x": x[c], "wt": wt, "bias": bias} for c in range(N_CORES)]
    res = run_bass_kernel_spmd(nc, in_maps, core_ids=list(range(N_CORES)))
    _cached["last_results"] = res
    y = np.empty((B, S, DOUT), dtype=np.float32)
    for c in range(N_CORES):
        y[c] = res.results[c]["yT"].T
    return y
